# revision 9
# baseline (speedup 1.0000x reference)
"""Trainium2 Bass kernel for batched windowed DFT (STFT-as-GEMM).

Problem: for each batch row of x (8, 262144), reflect-pad by 1024, frame into
513 overlapping windows (len 2048, hop 512), and multiply by dense Hann-windowed
sin/cos DFT matrices (2048x2048):  real = wcos @ frames^T, out = (real, -imag).

Strategy (one batch per NeuronCore, 8 cores): segment-DFT factorization.
Frames overlap 4x (hop 512, window 2048), so instead of the dense frame GEMM
(contraction 2048 per frame), transform each non-overlapping 512-sample segment
once against the 2048-bin basis:

    G_j[k] = sum_m xp[512j+m] e^{-2pi i k m/2048},   j = 0..515, k = 0..1023

(a 512-contraction GEMM -> 4x fewer FLOPs and 4x less weight traffic), then the
frame DFT is the exact linear recombination

    F_t[k] = sum_{q=0..3} (-i)^{kq} G_{t+q}[k]

with coefficients in {+-1, +-i}, and the Hann window is the exact 3-tap
frequency stencil  W[k] = 0.5 F[k] - 0.25(F[k-1] + F[k+1]).  The device does
the GEMM (the 99.9% of FLOPs) in fp16 with fp32 PSUM accumulation; the host
does only light linear marshalling: the recombination adds, the stencil, the
two edge rows k=1024/1025, and the Hermitian mirror to rows 1025..2047.
"""
import numpy as np

import concourse.bacc as bacc
import concourse.mybir as mybir
import concourse.tile as tile

F32 = mybir.dt.float32
F16 = mybir.dt.float16
SEG = 516        # number of 512-sample segments in the padded signal
NG = 16          # output tiles: 8 k-tiles (128 bins each) x {cos, sin}
N_CORES = 8


def build_nc(reps=1):
    nc = bacc.Bacc("TRN2", target_bir_lowering=False, debug=False,
                   num_devices=N_CORES)
    xs_d = nc.dram_tensor("xs", [128, 4 * SEG], F16, kind="ExternalInput")
    wt_d = nc.dram_tensor("wt", [128, NG * 512], F16, kind="ExternalInput")
    g_d = nc.dram_tensor("gout", [NG * 128, SEG], F16, kind="ExternalOutput")

    with tile.TileContext(nc) as tc:
        with (
            tc.tile_pool(name="xin", bufs=2) as xin,
            tc.tile_pool(name="wts", bufs=1) as wts,
            tc.tile_pool(name="gsb", bufs=6) as gsb,
            tc.tile_pool(name="ps", bufs=4, space="PSUM") as ps,
        ):
            Wg = [wts.tile([128, 2048], F16, tag=f"w{g}", name=f"w{g}")
                  for g in range(4)]

            for _rep in range(reps):
                XS = xin.tile([128, 4 * SEG], F16, tag="XS", name="XS")
                for half in range(2):
                    nc.scalar.dma_start(
                        XS[:, half * 2 * SEG:(half + 1) * 2 * SEG],
                        xs_d.ap()[:, half * 2 * SEG:(half + 1) * 2 * SEG])
                for w4 in range(4):
                    nc.gpsimd.dma_start(
                        Wg[w4][:], wt_d.ap()[:, w4 * 2048:w4 * 2048 + 2048])

                for g4 in range(4):
                    G16 = gsb.tile([128, 4 * SEG], F16, tag="g16", name="G16")
                    for sub in range(4):
                        g = g4 * 4 + sub
                        acc = ps.tile([128, SEG], F32, tag="acc")
                        for mq in range(4):
                            lhsT = Wg[g // 4][:, (g % 4) * 512 + mq * 128:
                                             (g % 4) * 512 + (mq + 1) * 128]
                            nc.tensor.matmul(acc[:, 0:512], lhsT,
                                             XS[:, mq * SEG:mq * SEG + 512],
                                             start=(mq == 0), stop=(mq == 3))
                            nc.tensor.matmul(acc[:, 512:SEG], lhsT,
                                             XS[:, mq * SEG + 512:mq * SEG + SEG],
                                             start=(mq == 0), stop=(mq == 3))
                        dst = G16[:, sub * SEG:(sub + 1) * SEG]
                        if g % 2 == 0:
                            nc.scalar.copy(dst, acc[:])
                        else:
                            nc.vector.tensor_copy(dst, acc[:])
                    nc.sync.dma_start(
                        g_d.ap()[g4 * 512:(g4 + 1) * 512, :].rearrange(
                            "(sub p) t -> p sub t", sub=4),
                        G16[:].rearrange("p (sub t) -> p sub t", sub=4))
    nc.compile()
    return nc


def _weights():
    """WT[p, g*512 + mq*128 + col]: lhsT blocks for the segment-DFT GEMM.

    Tile g = 2*h + comp covers bins k = 128h + col; contraction m = 128*mq + p.
    comp 0: cos(2 pi k m / 2048); comp 1: -sin(...)  (e^{-i theta} convention).
    """
    p = np.arange(128)
    out = np.empty((128, NG * 512), np.float32)
    for h in range(8):
        for comp in range(2):
            g = 2 * h + comp
            for mq in range(4):
                m = 128 * mq + p                      # (128,)
                k = 128 * h + np.arange(128)          # (128,)
                th = 2.0 * np.pi * np.outer(m, k) / 2048.0
                blk = np.cos(th) if comp == 0 else -np.sin(th)
                out[:, g * 512 + mq * 128:g * 512 + (mq + 1) * 128] = blk
    return out.astype(np.float16)


def host_prep(x, wsin=None, wcos=None):
    """Marshal full inputs into per-core input maps (pure data movement)."""
    x = np.asarray(x, dtype=np.float32)
    B = x.shape[0]
    xp = np.pad(x, ((0, 0), (1024, 1024)), mode="reflect")
    # XS[p, mq*SEG + j] = xp[512j + 128mq + p]
    XS = np.ascontiguousarray(
        xp.reshape(B, SEG, 4, 128).transpose(0, 3, 2, 1)
    ).reshape(B, 128, 4 * SEG).astype(np.float16)
    WT = _weights()
    ins = [{"xs": XS[b], "wt": WT} for b in range(B)]
    return ins, xp


def postprocess(gout, xp):
    """Recombination + Hann stencil + Hermitian mirror (host, light linear ops)."""
    B = gout.shape[0]
    # unpack device tiles: tile g=2h+comp rows -> Gre/Gim (B, 1024, SEG)
    go = np.ascontiguousarray(gout.reshape(B, 8, 2, 128, SEG)).astype(np.float32)
    G = np.empty((B, 1026, SEG), np.complex64)
    G[:, :1024] = (go[:, :, 0] + 1j * go[:, :, 1]).reshape(B, 1024, SEG)
    # edge rows k=1024, 1025 from the padded signal directly (tiny GEMM)
    seg = xp.reshape(B, SEG, 512)
    m = np.arange(512)
    alt = (-1.0) ** m
    v1025c = alt * np.cos(2 * np.pi * m / 2048.0)
    v1025s = alt * -np.sin(2 * np.pi * m / 2048.0)
    G[:, 1024] = (seg @ alt.astype(np.float32))
    G[:, 1025] = (seg @ v1025c.astype(np.float32)) + 1j * (seg @ v1025s.astype(np.float32))

    kk = np.arange(1026)
    tw = ((-1j) ** (kk % 4)).astype(np.complex64)[None, :, None]
    sgn = np.where(kk % 2 == 0, 1.0, -1.0).astype(np.float32)[None, :, None]
    D = G[:, :, 0:515] + tw * G[:, :, 1:516]
    F = D[:, :, 0:513] + sgn * D[:, :, 2:515]          # (B, 1026, 513)

    # stencil W[k] = 0.5F[k] - 0.25(F[k-1]+F[k+1]), k=0..1024; F[-1] = conj F[1]
    W = np.empty((B, 1025, 513), np.complex64)
    W[:, 1:] = 0.5 * F[:, 1:1025] - 0.25 * (F[:, 0:1024] + F[:, 2:1026])
    W[:, 0] = 0.5 * F[:, 0] - 0.25 * (np.conj(F[:, 1]) + F[:, 1])

    R = np.empty((B, 2048, 513), np.float32)
    I = np.empty((B, 2048, 513), np.float32)
    R[:, :1025] = W.real
    I[:, :1025] = W.imag
    R[:, 1025:] = W.real[:, 1023:0:-1]
    I[:, 1025:] = -W.imag[:, 1023:0:-1]
    return R, I


class _Runner:
    """Build once, jit once, run many (shard_map over the 8 cores)."""

    def __init__(self, reps=1):
        import jax
        from jax.sharding import Mesh, PartitionSpec
        from jax.experimental.shard_map import shard_map
        from concourse.bass2jax import _bass_exec_p, install_neuronx_cc_hook

        install_neuronx_cc_hook()
        self.jax = jax
        nc = build_nc(reps=reps)
        self.nc = nc
        in_names, out_names, out_avals = [], [], []
        for alloc in nc.m.functions[0].allocations:
            if not isinstance(alloc, mybir.MemoryLocationSet):
                continue
            name = alloc.memorylocations[0].name
            if alloc.kind == "ExternalInput":
                in_names.append(name)
            elif alloc.kind == "ExternalOutput":
                out_names.append(name)
                out_avals.append(jax.core.ShapedArray(
                    tuple(alloc.tensor_shape), mybir.dt.np(alloc.dtype)))
        self.in_names, self.out_names, self.out_avals = in_names, out_names, out_avals
        n_params = len(in_names)
        all_names = in_names + out_names

        def _body(*args):
            outs = _bass_exec_p.bind(
                *args,
                out_avals=tuple(out_avals),
                in_names=tuple(all_names),
                out_names=tuple(out_names),
                lowering_input_output_aliases=(),
                sim_require_finite=True,
                sim_require_nnan=True,
                nc=nc,
            )
            return tuple(outs)

        devices = jax.devices()[:N_CORES]
        mesh = Mesh(np.asarray(devices), ("core",))
        n_outs = len(out_names)
        self._fn = jax.jit(
            shard_map(_body, mesh=mesh,
                      in_specs=(PartitionSpec("core"),) * (n_params + n_outs),
                      out_specs=(PartitionSpec("core"),) * n_outs,
                      check_rep=False),
            keep_unused=True,
        )
        self._zeros = [np.zeros((N_CORES * a.shape[0], *a.shape[1:]), a.dtype)
                       for a in out_avals]

    def prepare(self, in_maps):
        pid = self.nc.partition_id_tensor.name if self.nc.partition_id_tensor else None
        in_maps = [
            dict(m, **({pid: np.array([[c]], dtype=np.uint32)} if pid else {}))
            for c, m in enumerate(in_maps)
        ]
        concat = [np.concatenate([np.asarray(m[name]) for m in in_maps], axis=0)
                  for name in self.in_names]
        self._args = [self.jax.device_put(a) for a in concat + self._zeros]
        self.jax.block_until_ready(self._args)

    def run(self):
        out = self._fn(*self._args)
        self.jax.block_until_ready(out)
        return out

    def results(self, out):
        # single output tensor "gout": (N_CORES*2048, SEG) -> (N_CORES, 2048, SEG)
        a = np.asarray(out[0])
        return a.reshape(N_CORES, NG * 128, SEG)


_RUNNER = None


def kernel(x, wsin, wcos):
    """Full inputs in, full output out: returns (real, -imag) as in reference."""
    global _RUNNER
    if _RUNNER is None:
        _RUNNER = _Runner(reps=1)
    ins, xp = host_prep(x)
    _RUNNER.prepare(ins)
    out = _RUNNER.run()
    gout = _RUNNER.results(out)
    return postprocess(gout, xp)


# revision 10
# speedup vs baseline: 1.0003x; 1.0003x over previous
"""Trainium2 Bass kernel for batched windowed DFT (STFT-as-GEMM).

Problem: for each batch row of x (8, 262144), reflect-pad by 1024, frame into
513 overlapping windows (len 2048, hop 512), and multiply by dense Hann-windowed
sin/cos DFT matrices (2048x2048):  real = wcos @ frames^T, out = (real, -imag).

Strategy (one batch per NeuronCore, 8 cores): segment-DFT factorization.
Frames overlap 4x (hop 512, window 2048), so instead of the dense frame GEMM
(contraction 2048 per frame), transform each non-overlapping 512-sample segment
once against the 2048-bin basis:

    G_j[k] = sum_m xp[512j+m] e^{-2pi i k m/2048},   j = 0..515, k = 0..1023

(a 512-contraction GEMM -> 4x fewer FLOPs and 4x less weight traffic), then the
frame DFT is the exact linear recombination

    F_t[k] = sum_{q=0..3} (-i)^{kq} G_{t+q}[k]

with coefficients in {+-1, +-i}, and the Hann window is the exact 3-tap
frequency stencil  W[k] = 0.5 F[k] - 0.25(F[k-1] + F[k+1]).  The device does
the GEMM (the 99.9% of FLOPs) in fp16 with fp32 PSUM accumulation; the host
does only light linear marshalling: the recombination adds, the stencil, the
two edge rows k=1024/1025, and the Hermitian mirror to rows 1025..2047.
"""
import numpy as np
import ml_dtypes

BF16 = ml_dtypes.bfloat16

import concourse.bacc as bacc
import concourse.mybir as mybir
import concourse.tile as tile

F32 = mybir.dt.float32
F16 = mybir.dt.bfloat16
SEG = 516        # number of 512-sample segments in the padded signal
NG = 16          # output tiles: 8 k-tiles (128 bins each) x {cos, sin}
N_CORES = 8


def build_nc(reps=1):
    nc = bacc.Bacc("TRN2", target_bir_lowering=False, debug=False,
                   num_devices=N_CORES)
    xs_d = nc.dram_tensor("xs", [128, 4 * SEG], F16, kind="ExternalInput")
    wt_d = nc.dram_tensor("wt", [128, NG * 512], F16, kind="ExternalInput")
    g_d = nc.dram_tensor("gout", [NG * 128, SEG], F16, kind="ExternalOutput")

    with tile.TileContext(nc) as tc:
        with (
            tc.tile_pool(name="xin", bufs=2) as xin,
            tc.tile_pool(name="wts", bufs=1) as wts,
            tc.tile_pool(name="gsb", bufs=6) as gsb,
            tc.tile_pool(name="ps", bufs=4, space="PSUM") as ps,
        ):
            Wg = [wts.tile([128, 2048], F16, tag=f"w{g}", name=f"w{g}")
                  for g in range(4)]

            for _rep in range(reps):
                XS = xin.tile([128, 4 * SEG], F16, tag="XS", name="XS")
                for half in range(2):
                    nc.scalar.dma_start(
                        XS[:, half * 2 * SEG:(half + 1) * 2 * SEG],
                        xs_d.ap()[:, half * 2 * SEG:(half + 1) * 2 * SEG])
                for w4 in range(4):
                    nc.gpsimd.dma_start(
                        Wg[w4][:], wt_d.ap()[:, w4 * 2048:w4 * 2048 + 2048])

                for g4 in range(4):
                    G16 = gsb.tile([128, 4 * SEG], F16, tag="g16", name="G16")
                    for sub in range(4):
                        g = g4 * 4 + sub
                        acc = ps.tile([128, SEG], F32, tag="acc")
                        for mq in range(4):
                            lhsT = Wg[g // 4][:, (g % 4) * 512 + mq * 128:
                                             (g % 4) * 512 + (mq + 1) * 128]
                            nc.tensor.matmul(acc[:, 0:512], lhsT,
                                             XS[:, mq * SEG:mq * SEG + 512],
                                             start=(mq == 0), stop=(mq == 3))
                            nc.tensor.matmul(acc[:, 512:SEG], lhsT,
                                             XS[:, mq * SEG + 512:mq * SEG + SEG],
                                             start=(mq == 0), stop=(mq == 3))
                        dst = G16[:, sub * SEG:(sub + 1) * SEG]
                        if g % 2 == 0:
                            nc.scalar.copy(dst, acc[:])
                        else:
                            nc.vector.tensor_copy(dst, acc[:])
                    nc.sync.dma_start(
                        g_d.ap()[g4 * 512:(g4 + 1) * 512, :].rearrange(
                            "(sub p) t -> p sub t", sub=4),
                        G16[:].rearrange("p (sub t) -> p sub t", sub=4))
    nc.compile()
    return nc


def _weights():
    """WT[p, g*512 + mq*128 + col]: lhsT blocks for the segment-DFT GEMM.

    Tile g = 2*h + comp covers bins k = 128h + col; contraction m = 128*mq + p.
    comp 0: cos(2 pi k m / 2048); comp 1: -sin(...)  (e^{-i theta} convention).
    """
    p = np.arange(128)
    out = np.empty((128, NG * 512), np.float32)
    for h in range(8):
        for comp in range(2):
            g = 2 * h + comp
            for mq in range(4):
                m = 128 * mq + p                      # (128,)
                k = 128 * h + np.arange(128)          # (128,)
                th = 2.0 * np.pi * np.outer(m, k) / 2048.0
                blk = np.cos(th) if comp == 0 else -np.sin(th)
                out[:, g * 512 + mq * 128:g * 512 + (mq + 1) * 128] = blk
    return out.astype(BF16)


def host_prep(x, wsin=None, wcos=None):
    """Marshal full inputs into per-core input maps (pure data movement)."""
    x = np.asarray(x, dtype=np.float32)
    B = x.shape[0]
    xp = np.pad(x, ((0, 0), (1024, 1024)), mode="reflect")
    # XS[p, mq*SEG + j] = xp[512j + 128mq + p]
    XS = np.ascontiguousarray(
        xp.reshape(B, SEG, 4, 128).transpose(0, 3, 2, 1)
    ).reshape(B, 128, 4 * SEG).astype(BF16)
    WT = _weights()
    ins = [{"xs": XS[b], "wt": WT} for b in range(B)]
    return ins, xp


def postprocess(gout, xp):
    """Recombination + Hann stencil + Hermitian mirror (host, light linear ops)."""
    B = gout.shape[0]
    # unpack device tiles: tile g=2h+comp rows -> Gre/Gim (B, 1024, SEG)
    go = np.ascontiguousarray(gout.reshape(B, 8, 2, 128, SEG)).astype(np.float32)
    G = np.empty((B, 1026, SEG), np.complex64)
    G[:, :1024] = (go[:, :, 0] + 1j * go[:, :, 1]).reshape(B, 1024, SEG)
    # edge rows k=1024, 1025 from the padded signal directly (tiny GEMM)
    seg = xp.reshape(B, SEG, 512)
    m = np.arange(512)
    alt = (-1.0) ** m
    v1025c = alt * np.cos(2 * np.pi * m / 2048.0)
    v1025s = alt * -np.sin(2 * np.pi * m / 2048.0)
    G[:, 1024] = (seg @ alt.astype(np.float32))
    G[:, 1025] = (seg @ v1025c.astype(np.float32)) + 1j * (seg @ v1025s.astype(np.float32))

    kk = np.arange(1026)
    tw = ((-1j) ** (kk % 4)).astype(np.complex64)[None, :, None]
    sgn = np.where(kk % 2 == 0, 1.0, -1.0).astype(np.float32)[None, :, None]
    D = G[:, :, 0:515] + tw * G[:, :, 1:516]
    F = D[:, :, 0:513] + sgn * D[:, :, 2:515]          # (B, 1026, 513)

    # stencil W[k] = 0.5F[k] - 0.25(F[k-1]+F[k+1]), k=0..1024; F[-1] = conj F[1]
    W = np.empty((B, 1025, 513), np.complex64)
    W[:, 1:] = 0.5 * F[:, 1:1025] - 0.25 * (F[:, 0:1024] + F[:, 2:1026])
    W[:, 0] = 0.5 * F[:, 0] - 0.25 * (np.conj(F[:, 1]) + F[:, 1])

    R = np.empty((B, 2048, 513), np.float32)
    I = np.empty((B, 2048, 513), np.float32)
    R[:, :1025] = W.real
    I[:, :1025] = W.imag
    R[:, 1025:] = W.real[:, 1023:0:-1]
    I[:, 1025:] = -W.imag[:, 1023:0:-1]
    return R, I


class _Runner:
    """Build once, jit once, run many (shard_map over the 8 cores)."""

    def __init__(self, reps=1):
        import jax
        from jax.sharding import Mesh, PartitionSpec
        from jax.experimental.shard_map import shard_map
        from concourse.bass2jax import _bass_exec_p, install_neuronx_cc_hook

        install_neuronx_cc_hook()
        self.jax = jax
        nc = build_nc(reps=reps)
        self.nc = nc
        in_names, out_names, out_avals = [], [], []
        for alloc in nc.m.functions[0].allocations:
            if not isinstance(alloc, mybir.MemoryLocationSet):
                continue
            name = alloc.memorylocations[0].name
            if alloc.kind == "ExternalInput":
                in_names.append(name)
            elif alloc.kind == "ExternalOutput":
                out_names.append(name)
                out_avals.append(jax.core.ShapedArray(
                    tuple(alloc.tensor_shape), mybir.dt.np(alloc.dtype)))
        self.in_names, self.out_names, self.out_avals = in_names, out_names, out_avals
        n_params = len(in_names)
        all_names = in_names + out_names

        def _body(*args):
            outs = _bass_exec_p.bind(
                *args,
                out_avals=tuple(out_avals),
                in_names=tuple(all_names),
                out_names=tuple(out_names),
                lowering_input_output_aliases=(),
                sim_require_finite=True,
                sim_require_nnan=True,
                nc=nc,
            )
            return tuple(outs)

        devices = jax.devices()[:N_CORES]
        mesh = Mesh(np.asarray(devices), ("core",))
        n_outs = len(out_names)
        self._fn = jax.jit(
            shard_map(_body, mesh=mesh,
                      in_specs=(PartitionSpec("core"),) * (n_params + n_outs),
                      out_specs=(PartitionSpec("core"),) * n_outs,
                      check_rep=False),
            keep_unused=True,
        )
        self._zeros = [np.zeros((N_CORES * a.shape[0], *a.shape[1:]), a.dtype)
                       for a in out_avals]

    def prepare(self, in_maps):
        pid = self.nc.partition_id_tensor.name if self.nc.partition_id_tensor else None
        in_maps = [
            dict(m, **({pid: np.array([[c]], dtype=np.uint32)} if pid else {}))
            for c, m in enumerate(in_maps)
        ]
        concat = [np.concatenate([np.asarray(m[name]) for m in in_maps], axis=0)
                  for name in self.in_names]
        self._args = [self.jax.device_put(a) for a in concat + self._zeros]
        self.jax.block_until_ready(self._args)

    def run(self):
        out = self._fn(*self._args)
        self.jax.block_until_ready(out)
        return out

    def results(self, out):
        # single output tensor "gout": (N_CORES*2048, SEG) -> (N_CORES, 2048, SEG)
        a = np.asarray(out[0])
        return a.reshape(N_CORES, NG * 128, SEG)


_RUNNER = None


def kernel(x, wsin, wcos):
    """Full inputs in, full output out: returns (real, -imag) as in reference."""
    global _RUNNER
    if _RUNNER is None:
        _RUNNER = _Runner(reps=1)
    ins, xp = host_prep(x)
    _RUNNER.prepare(ins)
    out = _RUNNER.run()
    gout = _RUNNER.results(out)
    return postprocess(gout, xp)


# revision 11
# speedup vs baseline: 1.0245x; 1.0243x over previous
"""Trainium2 Bass kernel for batched windowed DFT (STFT-as-GEMM).

Problem: for each batch row of x (8, 262144), reflect-pad by 1024, frame into
513 overlapping windows (len 2048, hop 512), and multiply by dense Hann-windowed
sin/cos DFT matrices (2048x2048):  real = wcos @ frames^T, out = (real, -imag).

Strategy (one batch per NeuronCore, 8 cores): segment-DFT factorization.
Frames overlap 4x (hop 512, window 2048), so instead of the dense frame GEMM
(contraction 2048 per frame), transform each non-overlapping 512-sample segment
once against the 2048-bin basis:

    G_j[k] = sum_m xp[512j+m] e^{-2pi i k m/2048},   j = 0..515, k = 0..1023

(a 512-contraction GEMM -> 4x fewer FLOPs and 4x less weight traffic), then the
frame DFT is the exact linear recombination

    F_t[k] = sum_{q=0..3} (-i)^{kq} G_{t+q}[k]

with coefficients in {+-1, +-i}, and the Hann window is the exact 3-tap
frequency stencil  W[k] = 0.5 F[k] - 0.25(F[k-1] + F[k+1]).  The device does
the GEMM (the 99.9% of FLOPs) in fp16 with fp32 PSUM accumulation; the host
does only light linear marshalling: the recombination adds, the stencil, the
two edge rows k=1024/1025, and the Hermitian mirror to rows 1025..2047.
"""
import numpy as np
import ml_dtypes

BF16 = ml_dtypes.bfloat16

import concourse.bacc as bacc
import concourse.mybir as mybir
import concourse.tile as tile

F32 = mybir.dt.float32
F16 = mybir.dt.bfloat16
SEG = 516        # number of 512-sample segments in the padded signal
NG = 16          # output tiles: 8 k-tiles (128 bins each) x {cos, sin}
N_CORES = 8


def build_nc(reps=1):
    nc = bacc.Bacc("TRN2", target_bir_lowering=False, debug=False,
                   num_devices=N_CORES)
    xs_d = nc.dram_tensor("xs", [128, 4 * SEG], F16, kind="ExternalInput")
    wt_d = nc.dram_tensor("wt", [128, NG * 512], F16, kind="ExternalInput")
    g_d = nc.dram_tensor("gout", [NG * 128, SEG], F16, kind="ExternalOutput")

    with tile.TileContext(nc) as tc:
        with (
            tc.tile_pool(name="xin", bufs=2) as xin,
            tc.tile_pool(name="wts", bufs=1) as wts,
            tc.tile_pool(name="gsb", bufs=6) as gsb,
            tc.tile_pool(name="ps", bufs=4, space="PSUM") as ps,
        ):
            Wg = [wts.tile([128, 2048], F16, tag=f"w{g}", name=f"w{g}")
                  for g in range(4)]

            for _rep in range(reps):
                XS = xin.tile([128, 4 * SEG], F16, tag="XS", name="XS")
                for half in range(2):
                    nc.scalar.dma_start(
                        XS[:, half * 2 * SEG:(half + 1) * 2 * SEG],
                        xs_d.ap()[:, half * 2 * SEG:(half + 1) * 2 * SEG])
                for w4 in range(4):
                    nc.gpsimd.dma_start(
                        Wg[w4][:], wt_d.ap()[:, w4 * 2048:w4 * 2048 + 2048])

                for g4 in range(4):
                    G16 = gsb.tile([128, 4 * SEG], F16, tag="g16", name="G16")
                    for sub in range(4):
                        g = g4 * 4 + sub
                        acc = ps.tile([128, SEG], F32, tag="acc")
                        for mq in range(4):
                            lhsT = Wg[g // 4][:, (g % 4) * 512 + mq * 128:
                                             (g % 4) * 512 + (mq + 1) * 128]
                            nc.tensor.matmul(acc[:, 0:256], lhsT,
                                             XS[:, mq * SEG:mq * SEG + 256],
                                             start=(mq == 0), stop=(mq == 3))
                            nc.tensor.matmul(acc[:, 256:512], lhsT,
                                             XS[:, mq * SEG + 256:mq * SEG + 512],
                                             start=(mq == 0), stop=(mq == 3))
                            nc.tensor.matmul(acc[:, 512:SEG], lhsT,
                                             XS[:, mq * SEG + 512:mq * SEG + SEG],
                                             start=(mq == 0), stop=(mq == 3))
                        dst = G16[:, sub * SEG:(sub + 1) * SEG]
                        if g % 2 == 0:
                            nc.scalar.copy(dst, acc[:])
                        else:
                            nc.vector.tensor_copy(dst, acc[:])
                    nc.sync.dma_start(
                        g_d.ap()[g4 * 512:(g4 + 1) * 512, :].rearrange(
                            "(sub p) t -> p sub t", sub=4),
                        G16[:].rearrange("p (sub t) -> p sub t", sub=4))
    nc.compile()
    return nc


def _weights():
    """WT[p, g*512 + mq*128 + col]: lhsT blocks for the segment-DFT GEMM.

    Tile g = 2*h + comp covers bins k = 128h + col; contraction m = 128*mq + p.
    comp 0: cos(2 pi k m / 2048); comp 1: -sin(...)  (e^{-i theta} convention).
    """
    p = np.arange(128)
    out = np.empty((128, NG * 512), np.float32)
    for h in range(8):
        for comp in range(2):
            g = 2 * h + comp
            for mq in range(4):
                m = 128 * mq + p                      # (128,)
                k = 128 * h + np.arange(128)          # (128,)
                th = 2.0 * np.pi * np.outer(m, k) / 2048.0
                blk = np.cos(th) if comp == 0 else -np.sin(th)
                out[:, g * 512 + mq * 128:g * 512 + (mq + 1) * 128] = blk
    return out.astype(BF16)


def host_prep(x, wsin=None, wcos=None):
    """Marshal full inputs into per-core input maps (pure data movement)."""
    x = np.asarray(x, dtype=np.float32)
    B = x.shape[0]
    xp = np.pad(x, ((0, 0), (1024, 1024)), mode="reflect")
    # XS[p, mq*SEG + j] = xp[512j + 128mq + p]
    XS = np.ascontiguousarray(
        xp.reshape(B, SEG, 4, 128).transpose(0, 3, 2, 1)
    ).reshape(B, 128, 4 * SEG).astype(BF16)
    WT = _weights()
    ins = [{"xs": XS[b], "wt": WT} for b in range(B)]
    return ins, xp


def postprocess(gout, xp):
    """Recombination + Hann stencil + Hermitian mirror (host, light linear ops)."""
    B = gout.shape[0]
    # unpack device tiles: tile g=2h+comp rows -> Gre/Gim (B, 1024, SEG)
    go = np.ascontiguousarray(gout.reshape(B, 8, 2, 128, SEG)).astype(np.float32)
    G = np.empty((B, 1026, SEG), np.complex64)
    G[:, :1024] = (go[:, :, 0] + 1j * go[:, :, 1]).reshape(B, 1024, SEG)
    # edge rows k=1024, 1025 from the padded signal directly (tiny GEMM)
    seg = xp.reshape(B, SEG, 512)
    m = np.arange(512)
    alt = (-1.0) ** m
    v1025c = alt * np.cos(2 * np.pi * m / 2048.0)
    v1025s = alt * -np.sin(2 * np.pi * m / 2048.0)
    G[:, 1024] = (seg @ alt.astype(np.float32))
    G[:, 1025] = (seg @ v1025c.astype(np.float32)) + 1j * (seg @ v1025s.astype(np.float32))

    kk = np.arange(1026)
    tw = ((-1j) ** (kk % 4)).astype(np.complex64)[None, :, None]
    sgn = np.where(kk % 2 == 0, 1.0, -1.0).astype(np.float32)[None, :, None]
    D = G[:, :, 0:515] + tw * G[:, :, 1:516]
    F = D[:, :, 0:513] + sgn * D[:, :, 2:515]          # (B, 1026, 513)

    # stencil W[k] = 0.5F[k] - 0.25(F[k-1]+F[k+1]), k=0..1024; F[-1] = conj F[1]
    W = np.empty((B, 1025, 513), np.complex64)
    W[:, 1:] = 0.5 * F[:, 1:1025] - 0.25 * (F[:, 0:1024] + F[:, 2:1026])
    W[:, 0] = 0.5 * F[:, 0] - 0.25 * (np.conj(F[:, 1]) + F[:, 1])

    R = np.empty((B, 2048, 513), np.float32)
    I = np.empty((B, 2048, 513), np.float32)
    R[:, :1025] = W.real
    I[:, :1025] = W.imag
    R[:, 1025:] = W.real[:, 1023:0:-1]
    I[:, 1025:] = -W.imag[:, 1023:0:-1]
    return R, I


class _Runner:
    """Build once, jit once, run many (shard_map over the 8 cores)."""

    def __init__(self, reps=1):
        import jax
        from jax.sharding import Mesh, PartitionSpec
        from jax.experimental.shard_map import shard_map
        from concourse.bass2jax import _bass_exec_p, install_neuronx_cc_hook

        install_neuronx_cc_hook()
        self.jax = jax
        nc = build_nc(reps=reps)
        self.nc = nc
        in_names, out_names, out_avals = [], [], []
        for alloc in nc.m.functions[0].allocations:
            if not isinstance(alloc, mybir.MemoryLocationSet):
                continue
            name = alloc.memorylocations[0].name
            if alloc.kind == "ExternalInput":
                in_names.append(name)
            elif alloc.kind == "ExternalOutput":
                out_names.append(name)
                out_avals.append(jax.core.ShapedArray(
                    tuple(alloc.tensor_shape), mybir.dt.np(alloc.dtype)))
        self.in_names, self.out_names, self.out_avals = in_names, out_names, out_avals
        n_params = len(in_names)
        all_names = in_names + out_names

        def _body(*args):
            outs = _bass_exec_p.bind(
                *args,
                out_avals=tuple(out_avals),
                in_names=tuple(all_names),
                out_names=tuple(out_names),
                lowering_input_output_aliases=(),
                sim_require_finite=True,
                sim_require_nnan=True,
                nc=nc,
            )
            return tuple(outs)

        devices = jax.devices()[:N_CORES]
        mesh = Mesh(np.asarray(devices), ("core",))
        n_outs = len(out_names)
        self._fn = jax.jit(
            shard_map(_body, mesh=mesh,
                      in_specs=(PartitionSpec("core"),) * (n_params + n_outs),
                      out_specs=(PartitionSpec("core"),) * n_outs,
                      check_rep=False),
            keep_unused=True,
        )
        self._zeros = [np.zeros((N_CORES * a.shape[0], *a.shape[1:]), a.dtype)
                       for a in out_avals]

    def prepare(self, in_maps):
        pid = self.nc.partition_id_tensor.name if self.nc.partition_id_tensor else None
        in_maps = [
            dict(m, **({pid: np.array([[c]], dtype=np.uint32)} if pid else {}))
            for c, m in enumerate(in_maps)
        ]
        concat = [np.concatenate([np.asarray(m[name]) for m in in_maps], axis=0)
                  for name in self.in_names]
        self._args = [self.jax.device_put(a) for a in concat + self._zeros]
        self.jax.block_until_ready(self._args)

    def run(self):
        out = self._fn(*self._args)
        self.jax.block_until_ready(out)
        return out

    def results(self, out):
        # single output tensor "gout": (N_CORES*2048, SEG) -> (N_CORES, 2048, SEG)
        a = np.asarray(out[0])
        return a.reshape(N_CORES, NG * 128, SEG)


_RUNNER = None


def kernel(x, wsin, wcos):
    """Full inputs in, full output out: returns (real, -imag) as in reference."""
    global _RUNNER
    if _RUNNER is None:
        _RUNNER = _Runner(reps=1)
    ins, xp = host_prep(x)
    _RUNNER.prepare(ins)
    out = _RUNNER.run()
    gout = _RUNNER.results(out)
    return postprocess(gout, xp)
